# revision 61
# baseline (speedup 1.0000x reference)
"""Trainium2 Bass kernel for nn_Beltrami (retrieval_knn).

Per-core (batch-parallel over 8 cores): fc (f32r hi/lo-split matmuls for exact
pos; hi-only for feat and row norms) -> normalize pos -> quantize posT to fp16
-> cosine sim via fp16 matmul -> exp on Act (PSUM->SBUF, E32 f32, doubles as
the PSUM-freeing copy) -> top-32 via 16x max8 over 256-col chunks + 4-round
max8/match_replace refine keeping all 32 values (r32). The mask work is then
spread across engines so the saturated DVE only touches 4096 cols once more:
Act compares in f32 (M16 = Relu(1e11*E + bias) saturating to {0,inf} f16),
Pool copies E32->E16, DVE applies A = min(M16, E16) as an all-f16 2x-mode TT,
and the softmax denominator comes free as sum(r32). Then SBUF->SBUF DMA
transpose of A -> A@feat fp16 gather matmul (3 tiles behind the sim) ->
unnormalized rows + per-tile asum to DRAM; the host does the final divide.
"""
import sys
import numpy as np

sys.path.insert(0, "/opt/trn_rl_repo")

B, N, C, K = 8, 4096, 256, 32
NT = N // 128          # 32 query tiles of 128 rows
NEG = -1.0e30

_CACHE = {}


def _build(reps=1):
    from contextlib import ExitStack
    import concourse.bass as bass
    import concourse.bacc as bacc
    import concourse.tile as tile
    from concourse import mybir

    f32 = mybir.dt.float32
    f32r = mybir.dt.float32r
    f16 = mybir.dt.float16
    AF = mybir.ActivationFunctionType
    Alu = mybir.AluOpType

    nc = bacc.Bacc("TRN2", target_bir_lowering=False, debug=False, num_devices=8)

    xT_in = nc.declare_dram_parameter("xT", [C, N], f32, isOutput=False)
    wT_in = nc.declare_dram_parameter("wT", [C, 2 * C], f32, isOutput=False)
    bf_in = nc.declare_dram_parameter("bf", [1, 2 * C], f32, isOutput=False)
    bp_in = nc.declare_dram_parameter("bp", [128, 2], f32, isOutput=False)
    id_in = nc.declare_dram_parameter("ident", [128, 128], f32, isOutput=False)
    out_p = nc.declare_dram_parameter("out", [N, C], f32, isOutput=True)
    asum_p = nc.declare_dram_parameter("asum", [128, NT], f32, isOutput=True)
    s_dram = nc.dram_tensor("s_scratch", [NT, 128], f32)

    with tile.TileContext(nc) as tc, ExitStack() as ctx:
        # ---------------- persistent pools ----------------
        persist = ctx.enter_context(tc.tile_pool(name="persist", bufs=1))
        featx_pool = ctx.enter_context(tc.tile_pool(name="featx", bufs=NT))

        # fp16 normalized posT, the only sim operand kept resident
        post16 = [persist.tile([128, N], f16, tag=f"post16_{ct}", name=f"post16_{ct}")
                  for ct in range(2)]
        asum_all = persist.tile([128, NT], f32, tag="asum_all")
        featx = [featx_pool.tile([128, C], f16, tag="featx", name=f"featx{i}") for i in range(NT)]
        nrm2 = persist.tile([128, NT], f32, tag="nrm2")
        s_til = persist.tile([128, NT], f32, tag="s_til")
        xt_r = [persist.tile([128, N], f32r, tag=f"xt_r{ct}", name=f"xt_r{ct}") for ct in range(2)]
        wt_r = [persist.tile([128, C], f32r, tag=f"wt_r{ct}", name=f"wt_r{ct}") for ct in range(2)]
        bf1_r = persist.tile([1, 2 * C], f32r, tag="bf1_r")
        ones1_r = persist.tile([1, 128], f32r, tag="ones1_r")

        # ---------------- startup: fc + normalize ----------------
        with ExitStack() as sctx:
            sb = sctx.enter_context(tc.tile_pool(name="start_sb", bufs=1))
            ps_fc = sctx.enter_context(tc.tile_pool(name="ps_fc", bufs=3, space="PSUM"))
            ps_pp = sctx.enter_context(tc.tile_pool(name="ps_pp", bufs=3, space="PSUM"))
            ps_tp = sctx.enter_context(tc.tile_pool(name="ps_tp", bufs=1, space="PSUM"))

            xstage_pool = sctx.enter_context(tc.tile_pool(name="xstage_pool", bufs=6))
            wt = [sb.tile([128, 2 * C], f32, tag=f"wt{ct}", name=f"wt{ct}") for ct in range(2)]
            xt_lo = [sb.tile([128, N], f32r, tag=f"xt_lo{ct}", name=f"xt_lo{ct}") for ct in range(2)]
            wph = [sb.tile([128, C], f32r, tag=f"wph{ct}", name=f"wph{ct}") for ct in range(2)]
            bf1 = sb.tile([1, 2 * C], f32, tag="bf1")
            bp = sb.tile([128, 2], f32, tag="bp")
            ident = sb.tile([128, 128], f32, tag="ident")
            ones1 = sb.tile([1, 128], f32, tag="ones1")
            scrap = sb.tile([128, C], f16, tag="scrap")
            post_raw = [sb.tile([128, N], f32, tag=f"post_raw{ct}", name=f"post_raw{ct}") for ct in range(2)]

            for ct in range(2):
                nc.sync.dma_start(wt[ct][:], wT_in[ct * 128:(ct + 1) * 128, :])
            nc.sync.dma_start(bf1[:], bf_in[:])
            nc.sync.dma_start(bp[:], bp_in[:])
            nc.sync.dma_start(ident[:], id_in[:])
            nc.vector.memset(ones1[:], 1.0)

            # PE p-state warmup: ~3.5us of junk matmuls while x stages, so
            # the fc/posT matmuls below run at full clock (ramp needs 3us
            # of continuous execution)
            junk16 = sb.tile([128, 128], f16, tag="junk16")
            nc.vector.memset(junk16[:], 1.0)
            jp = ps_tp.tile([128, 128], f32, tag="jp")
            for w in range(56):
                nc.tensor.matmul(jp[:], junk16[:], junk16[:],
                                 start=True, stop=True)
            for ct in range(2):
                nc.vector.tensor_copy(wt_r[ct][:], wt[ct][:, 0:C])
                nc.vector.tensor_copy(wph[ct][:], wt[ct][:, C:2 * C])
            nc.vector.tensor_copy(bf1_r[:], bf1[:])
            nc.vector.tensor_copy(ones1_r[:], ones1[:])

            # staging + fc interleaved per 512-chunk so PE stays dense
            def stage_chunk(ch):
                cs = slice(ch * 512, (ch + 1) * 512)
                for ct in range(2):
                    xstage = xstage_pool.tile([128, 512], f32, tag="xstage",
                                              name=f"xstage{ct}_{ch}")
                    nc.sync.dma_start(xstage[:], xT_in[ct * 128:(ct + 1) * 128, cs])
                    nc.vector.tensor_copy(xt_r[ct][:, cs], xstage[:])
                    nc.vector.tensor_tensor(xt_lo[ct][:, cs], xstage[:],
                                            xt_r[ct][:, cs], op=Alu.subtract)

            def posT_chunk(dt, ch):
                # W-residual (wpl) terms dropped: their ~5e-4 relative
                # contribution is at the same level as the later f16
                # quantization of pos, so they don't pay for their matmuls
                pp = ps_pp.tile([128, 512], f32, tag="pp", name=f"pp{dt}_{ch}")
                ds_ = slice(dt * 128, (dt + 1) * 128)
                cs_ = slice(ch * 512, (ch + 1) * 512)
                for ci, (lh, rh) in enumerate(
                        [(wph[0], xt_r[0]), (wph[0], xt_lo[0]),
                         (wph[1], xt_r[1]), (wph[1], xt_lo[1])]):
                    nc.tensor.matmul(pp[:], lh[:, ds_], rh[:, cs_],
                                     start=(ci == 0), stop=(ci == 3))
                # alternate the PSUM drain between Act and DVE: Act is the
                # startup bottleneck (squares + copies), DVE is idle here
                if ch == 0:
                    nc.scalar.activation(
                        post_raw[dt][:, ch * 512:(ch + 1) * 512], pp[:],
                        AF.Identity, bias=bp[:, dt:dt + 1])
                else:
                    nc.vector.tensor_scalar(
                        post_raw[dt][:, ch * 512:(ch + 1) * 512], pp[:],
                        bp[:, dt:dt + 1], scalar2=None, op0=Alu.add)

            for ch in range(8):
                stage_chunk(ch)
            for nt in range(NT):
                # hi-only pos fc: nrm2 tolerates ~1e-3 relative error (a
                # norm error scales a whole sim row, never reordering it).
                # feat fc is deferred into the loop's gather-free PE slots.
                fc = ps_fc.tile([128, C], f32, tag="fc")
                ns = slice(nt * 128, (nt + 1) * 128)
                nc.tensor.matmul(fc[:], xt_r[0][:, ns], wph[0][:],
                                 start=True, stop=False)
                nc.tensor.matmul(fc[:], xt_r[1][:, ns], wph[1][:],
                                 start=False, stop=False)
                nc.tensor.matmul(fc[:], ones1_r[:], bf1_r[:, C:2 * C],
                                 start=False, stop=True)
                nc.scalar.activation(scrap[:], fc[:], AF.Square,
                                     accum_out=nrm2[:, nt:nt + 1])

            # rsqrt entirely on DVE (no Act tables => no mid-kernel table
            # reloads): exact 1/x, linear sqrt seed, 2 Newton steps
            r0 = sb.tile([128, NT], f32, tag="r0")
            u = sb.tile([128, NT], f32, tag="u")
            nc.vector.reciprocal(r0[:], nrm2[:])
            nc.vector.tensor_scalar(s_til[:], r0[:], 7.423, scalar2=0.0305,
                                    op0=Alu.mult, op1=Alu.add)
            for _ in range(2):
                nc.vector.tensor_tensor(u[:], s_til[:], s_til[:], op=Alu.mult)
                nc.vector.tensor_tensor(u[:], u[:], nrm2[:], op=Alu.mult)
                nc.vector.tensor_scalar(u[:], u[:], -0.5, scalar2=1.5,
                                        op0=Alu.mult, op1=Alu.add)
                nc.vector.tensor_tensor(s_til[:], s_til[:], u[:], op=Alu.mult)

            # first pos chunks go ahead of the s-transpose so the PE does
            # not stall waiting for the DVE newton chain
            for ch in range(3):
                posT_chunk(0, ch)
                posT_chunk(1, ch)

            # transpose s [128, NT] -> [NT, 128], bounce via DRAM, broadcast-load
            st_ps = ps_tp.tile([NT, 128], f32, tag="st_ps")
            nc.tensor.transpose(st_ps[:], s_til[:], ident[:])
            stt = sb.tile([NT, 128], f32, tag="stt")
            nc.vector.tensor_copy(stt[:], st_ps[:])
            nc.sync.dma_start(s_dram[:], stt[:])

            # remaining pos chunks + scale-to-f16 streamed per chunk
            for ch in range(8):
                if ch >= 3:
                    posT_chunk(0, ch)
                    posT_chunk(1, ch)
                cs = slice(ch * 512, (ch + 1) * 512)
                sbc = xstage_pool.tile([128, 512], f32, tag="sbc",
                                       name=f"sbc{ch}")
                nc.sync.dma_start(
                    sbc[:], s_dram[:].flatten()[cs].partition_broadcast(128))
                # split the quantize tail across DVE and the idle Pool
                # engine so the first sim tiles unblock sooner
                nc.vector.tensor_tensor(post16[0][:, cs], post_raw[0][:, cs],
                                        sbc[:], op=Alu.mult)
                eng = nc.gpsimd if ch < 5 else nc.vector
                eng.tensor_tensor(post16[1][:, cs], post_raw[1][:, cs],
                                  sbc[:], op=Alu.mult)

        # ---------------- steady loop over query tiles ----------------
        e_pool = ctx.enter_context(tc.tile_pool(name="e_sb", bufs=2))
        e16_pool = ctx.enter_context(tc.tile_pool(name="e16_sb", bufs=3))
        m16_pool = ctx.enter_context(tc.tile_pool(name="m16_sb", bufs=2))
        a_pool = ctx.enter_context(tc.tile_pool(name="a_sb", bufs=2))
        at_pool = ctx.enter_context(tc.tile_pool(name="at_sb", bufs=4))
        osb_pool = ctx.enter_context(tc.tile_pool(name="osb_sb", bufs=2))
        cands_pool = ctx.enter_context(tc.tile_pool(name="cands_sb", bufs=2))
        r32_pool = ctx.enter_context(tc.tile_pool(name="r32_sb", bufs=2))
        bias_pool = ctx.enter_context(tc.tile_pool(name="bias_sb", bufs=2))
        ps_sim = ctx.enter_context(tc.tile_pool(name="ps_sim", bufs=2, space="PSUM"))
        ps_oe = ctx.enter_context(tc.tile_pool(name="ps_oe", bufs=2, space="PSUM"))
        ps_fe = ctx.enter_context(tc.tile_pool(name="ps_fe", bufs=2, space="PSUM"))

        def emit_feat(nt):
            fcf = ps_fe.tile([128, C], f32, tag="fcf", name=f"fcf{nt}")
            ns = slice(nt * 128, (nt + 1) * 128)
            nc.tensor.matmul(fcf[:], xt_r[0][:, ns], wt_r[0][:],
                             start=True, stop=False)
            nc.tensor.matmul(fcf[:], xt_r[1][:, ns], wt_r[1][:],
                             start=False, stop=False)
            nc.tensor.matmul(fcf[:], ones1_r[:], bf1_r[:, 0:C],
                             start=False, stop=True)
            nc.scalar.activation(featx[nt][:], fcf[:], AF.Copy)

        PIPE = 3  # gather matmuls lag the sim by 3 tiles (selection+mask+DMA)

        front = {}   # T -> (E32, cands)
        sel = {}     # T -> (E32, r32, bias32, rz)
        masks = {}   # T -> (M16, E16, rz)
        state = {}   # T -> (AT, rz, oe)

        def emit_front_half(T, half):
            qs = slice(T * 128, (T + 1) * 128)
            if half == 0:
                E32 = e_pool.tile([128, N], f32, tag="E32", name=f"E32_{T}")
                cands = cands_pool.tile([128, 128], f32, tag="cands")
                front[T] = (E32, cands)
            E32, cands = front[T]
            # 2-pass fp16 sim (stationary shared across 4 moving chunks to
            # coalesce weight loads) into 2-bank PSUM tiles, one exp per
            # 1024 cols (halves the Act access-latency overhead), max8
            # cands per 256 cols
            sms = [ps_sim.tile([128, 1024], f32, tag="sm",
                               name=f"sm{T}_{half}_{p}") for p in range(2)]
            for ct in range(2):
                for r in range(4):
                    o = half * 2048 + r * 512
                    nc.tensor.matmul(sms[r // 2][:, (r % 2) * 512:(r % 2) * 512 + 512],
                                     post16[ct][:, qs], post16[ct][:, o:o + 512],
                                     start=(ct == 0), stop=(ct == 1))
            for p in range(2):
                o = half * 2048 + p * 1024
                nc.scalar.activation(E32[:, o:o + 1024], sms[p][:], AF.Exp)
                for c in range(4):
                    gc = (half * 2 + p) * 4 + c
                    nc.vector.max(cands[:, gc * 8:(gc + 1) * 8],
                                  E32[:, gc * 256:(gc + 1) * 256])

        def emit_selection(T):
            E32, cands = front[T]
            # exact top-32 in exp space: 4 rounds of max8+replace, keeping
            # every round's 8 values -> r32 holds the full top-32 multiset
            r32 = r32_pool.tile([128, 32], f32, tag="r32")
            for rnd in range(4):
                nc.vector.max(r32[:, rnd * 8:(rnd + 1) * 8], cands[:])
                if rnd < 3:
                    nc.vector.match_replace(out=cands[:],
                                            in_to_replace=r32[:, rnd * 8:(rnd + 1) * 8],
                                            in_values=cands[:], imm_value=NEG)
            # mask bias for the Act compare: -SC*(v32 - 1e-6)
            bias32 = bias_pool.tile([128, 1], f32, tag="bias32")
            nc.vector.tensor_scalar(bias32[:], r32[:, 31:32], -1.0e11,
                                    scalar2=1.0e5, op0=Alu.mult, op1=Alu.add)
            # softmax denominator = sum of the selected 32 exp values;
            # exported to DRAM, normalization happens on the host
            nc.vector.tensor_reduce(asum_all[:, T:T + 1], r32[:], op=Alu.add,
                                    axis=mybir.AxisListType.XYZW)
            sel[T] = (E32, bias32)

        def emit_e16(T):
            # f16 copy of E for the cheap masked multiply; Pool is idle
            E32, _cands = front.pop(T)
            E16 = e16_pool.tile([128, N], f16, tag="E16", name=f"E16_{T}")
            nc.gpsimd.tensor_copy(E16[:], E32[:])
            masks[T] = [None, E16]

        def emit_mask(T):
            # f32-precision compare on Act: M = Relu(SC*E + SC*(1e-6 - v32))
            # saturates to {0, +inf} in the f16 output (E > 0 always)
            E32, bias32 = sel.pop(T)
            M16 = m16_pool.tile([128, N], f16, tag="M16", name=f"M16_{T}")
            nc.scalar.activation(M16[:], E32[:], AF.Relu,
                                 bias=bias32[:, 0:1], scale=1.0e11)
            masks[T][0] = M16

        def emit_apply(T):
            # A = min(M, E16): E16 where selected, 0 elsewhere (all-f16 2x TT)
            M16, E16 = masks.pop(T)
            A = a_pool.tile([128, N], f16, tag="A")
            nc.vector.tensor_tensor(A[:], M16[:], E16[:], op=Alu.min)
            # blocked transpose, SBUF -> SBUF (no DRAM bounce)
            AT = at_pool.tile([128, NT, 128], f16, tag="AT", name=f"AT_{T}")
            nc.sync.dma_start_transpose(AT[:], A[:])
            state[T] = [AT]

        def emit_back_half(T, half):
            if half == 0:
                oe = ps_oe.tile([128, C], f32, tag="oe", name=f"oe{T}")
                state[T].append(oe)
            AT, oe = state[T]
            for j in range(half * 16, half * 16 + 16):
                nc.tensor.matmul(oe[:], AT[:, j, :], featx[j][:],
                                 start=(j == 0), stop=(j == NT - 1))

        def emit_back_fin(T):
            AT, oe = state.pop(T)
            # unnormalized row sums: Act drains PSUM -> SBUF (gpsimd can't
            # read PSUM), host divides by the exported asum
            osb = osb_pool.tile([128, C], f32, tag="osb")
            nc.scalar.activation(osb[:], oe[:], AF.Copy)
            nc.sync.dma_start(out_p[T * 128:(T + 1) * 128, :], osb[:])

        for rep in range(reps):
            # the last tile's lagging stages are each pulled one round
            # earlier (its mask right after its selection, etc.), shaving a
            # full drain round off the pipeline tail
            for T in range(NT + PIPE - 1):
                for k in range(8):
                    nt = T * 8 + k
                    if nt < NT:
                        emit_feat(nt)
                # A-apply of T-2 first: its inputs are ready at round start,
                # so the DVE does useful work while tile T's exps land
                if 0 <= T - 2 < NT - 1:
                    emit_apply(T - 2)
                if T == NT:
                    emit_apply(NT - 1)
                # both sim halves first so the cand scan + refine complete
                # early in the round; the gather halves keep the PE busy
                # through the selection tail
                if T < NT:
                    emit_front_half(T, 0)
                    emit_front_half(T, 1)
                if T >= PIPE:
                    emit_back_half(T - PIPE, 0)
                    emit_back_half(T - PIPE, 1)
                if T == NT + PIPE - 2:
                    emit_back_half(NT - 1, 0)
                    emit_back_half(NT - 1, 1)
                if T < NT:
                    emit_selection(T)
                    emit_e16(T)
                if 0 <= T - 1 < NT - 1:
                    emit_mask(T - 1)
                if T == NT - 1:
                    emit_mask(T)
                if T >= PIPE:
                    emit_back_fin(T - PIPE)
                if T == NT + PIPE - 2:
                    emit_back_fin(NT - 1)
            nc.sync.dma_start(asum_p[:], asum_all[:])

    nc.compile()
    return nc


def kernel(x, W, bias, k):
    from concourse.bass_utils import run_bass_kernel_spmd

    x = np.asarray(x, dtype=np.float32)
    W = np.asarray(W, dtype=np.float32)
    bias = np.asarray(bias, dtype=np.float32)
    assert int(k) == K and x.shape == (B, N, C)

    if "nc" not in _CACHE:
        _CACHE["nc"] = _build()
    nc = _CACHE["nc"]

    wT = np.ascontiguousarray(W.T)                      # [C, 2C]
    bf = bias.reshape(1, 2 * C)
    bp = np.ascontiguousarray(
        bias[C:].reshape(2, 128).T)                     # [128, 2]
    ident = np.eye(128, dtype=np.float32)

    in_maps = []
    for b in range(B):
        xT = np.ascontiguousarray(x[b].T)               # [C, N]
        in_maps.append({"xT": xT, "wT": wT, "bf": bf, "bp": bp, "ident": ident})

    res = run_bass_kernel_spmd(nc, in_maps, list(range(B)))
    outs = []
    for b in range(B):
        o = res.results[b]["out"]                       # [N, C] unnormalized
        asum = res.results[b]["asum"]                   # [128, NT]
        denom = asum.T.reshape(N)                       # row T*128+r -> [T, r]
        outs.append(o / denom[:, None])
    return np.stack(outs, axis=0).astype(np.float32)

